# revision 2
# baseline (speedup 1.0000x reference)
"""Trainium2 Bass kernel for MultiHeadSelfAttentionModule, v2.

Sharding: core = 2*b + g (batch b, head-group g of 4 heads). Host reduces the
two head-groups' out-proj partials per batch and folds the exact bias terms
(see kernel v1 notes; the math rewrites are unchanged).

v2 layout/schedule changes vs v1:
  - exp tiles are 1024 wide (halves ACT per-instruction overhead; ACT is the
    near-bottleneck at ~133us of exp).
  - attention runs jj-outer (t-block) / head-inner so each t-block's
    out-projection can start right after its 4 heads finish, shrinking the
    epilogue tail to the last block only.
  - all Q/K/V projections are emitted as prologue/filler work; PE slack
    during the ACT-paced attention stream absorbs them.
  - softmax normalization: the V-ones row gives the denominator row in PSUM;
    DVE copies it to SBUF, GPSIMD partition_broadcast replicates it across 64
    partitions, and a single DVE tensor_tensor(divide) writes normalized
    context straight to ctxT. No PE broadcast matmuls, no extra PSUM bank.
  - LN stats: ACT Copy/Square+accum for the first tiles (ACT idles before the
    first exp), DVE bn_stats for the rest; rstd = (var+eps)^-0.5 via a single
    DVE tensor_scalar pow (ACT Sqrt no longer needed).
  - x.T transposes go through one [128,512] PSUM tile (4 PE transposes) and
    drain with a single strided DVE copy per x-tile.
  - out is written bf16 (halves the out DMA and the tail); host upcasts.

PSUM budget: ps_s 2x[128,1024] (4 banks) + ps_c 1x[65,1024] (2 banks) +
ps_mm 2x[128,512] (2 banks) = 8 banks exactly.
"""

import math
import sys

if "/opt/trn_rl_repo" not in sys.path:
    sys.path.insert(0, "/opt/trn_rl_repo")

import numpy as np

import concourse.bass as bass
import concourse.mybir as mybir
import concourse.tile as tile
from concourse.bass_utils import run_bass_kernel_spmd
from concourse.masks import make_identity

B, T, D = 4, 2048, 512
H, DK = 8, 64
HPC = 4  # heads per core
DO = HPC * DK  # per-core head dims = 256
N_CORES = 8
LN_EPS = 1e-5
F32 = mybir.dt.float32
F32R = mybir.dt.float32r
BF16 = mybir.dt.bfloat16
AF = mybir.ActivationFunctionType
ALU = mybir.AluOpType

N_TT = T // 128  # 16 t tiles
N_CS = D // 128  # 4 contraction slabs
N_IS = DO // 128  # 2 own-dim slabs
W = 1024  # exp block width
N_JJ = T // W  # 2
ACT_STATS = 10  # below this, even tiles' stats run on ACT (odd/rest: DVE bn)
PROJ_DEPRI = 300


def split_multi_waits(nc: bass.Bass) -> None:
    """Hoist all-but-one sync wait from every instruction onto injected
    single-wait NOPs on the same engine, immediately before the owner."""
    ctr = 0
    for fn in nc.m.functions:
        for bb in fn.blocks:
            insts = bb.instructions
            need = any(
                i.sync_info and i.sync_info.on_wait and len(i.sync_info.on_wait) > 1
                for i in insts
            )
            if not need:
                continue
            new = []
            for inst in insts:
                si = inst.sync_info
                if si and si.on_wait and len(si.on_wait) > 1:
                    waits = list(si.on_wait)
                    for w in waits[:-1]:
                        ctr += 1
                        nop = mybir.InstNoOp(
                            name=f"I-wsplit-{ctr}",
                            engine=inst.engine,
                            sync_info=mybir.SyncInfo(on_wait=[w], on_update=[]),
                        )
                        nc.register_instruction(nop)
                        new.append(nop)
                    si.on_wait = [waits[-1]]
                new.append(inst)
            bb.instructions = new


def build_nc() -> bass.Bass:
    nc = bass.Bass()

    xb = nc.declare_dram_parameter("xb", [T, D], BF16, isOutput=False)
    wqT = nc.declare_dram_parameter("wqT", [D, DO], BF16, isOutput=False)
    wkT = nc.declare_dram_parameter("wkT", [D, DO], BF16, isOutput=False)
    wvT = nc.declare_dram_parameter("wvT", [D, DO], BF16, isOutput=False)
    woT = nc.declare_dram_parameter("woT", [DO, D], BF16, isOutput=False)
    qb = nc.declare_dram_parameter("qb", [DO, 1], F32, isOutput=False)
    peT1 = nc.declare_dram_parameter("peT1", [DK, T], BF16, isOutput=False)
    out = nc.declare_dram_parameter("out", [T, D], BF16, isOutput=True)

    with tile.TileContext(nc) as tc:
        with (
            tc.tile_pool(name="persist", bufs=1) as persist,
            tc.tile_pool(name="xstream", bufs=16) as xstream,
            tc.tile_pool(name="lnscr", bufs=4) as lnscr,
            tc.tile_pool(name="lnst", bufs=16) as lnst,
            tc.tile_pool(name="lnw", bufs=6) as lnw,
            tc.tile_pool(name="expp", bufs=8) as expp,
            tc.tile_pool(name="denp", bufs=2) as denp,
            tc.tile_pool(name="pbp", bufs=2) as pbp,
            tc.tile_pool(name="outw", bufs=3) as outw,
            tc.tile_pool(name="pop", bufs=8) as pop,
            tc.tile_pool(name="ps_s", bufs=2, space="PSUM") as ps_s,
            tc.tile_pool(name="ps_c", bufs=3, space="PSUM") as ps_c,
            tc.tile_pool(name="ps_mm", bufs=1, space="PSUM") as ps_mm,
        ):
            xb_r = xb.rearrange("(n p) d -> p n d", p=128)
            x_tiles = [
                xstream.tile([128, D], BF16, tag="x", name=f"x_t{i}")
                for i in range(N_TT)
            ]
            for i in range(8):
                nc.sync.dma_start(out=x_tiles[i], in_=xb_r[:, i, :])

            # k/q weights + pe land right after the first 8 x tiles so the
            # i-slab-0 projections can start while x8-15 stream. pe is stored
            # [64, T] in DRAM and replicated to both partition halves (all
            # heads share it; both i-slabs read the same [128, T] tile).
            wkT_sb = persist.tile([128, N_CS, DO], BF16)
            nc.sync.dma_start(out=wkT_sb, in_=wkT.rearrange("(s p) i -> p s i", p=128))
            wqT_sb = persist.tile([128, N_CS, DO], BF16)
            nc.sync.dma_start(out=wqT_sb, in_=wqT.rearrange("(s p) i -> p s i", p=128))
            qb_sb = persist.tile([128, N_IS, 1], F32)
            nc.sync.dma_start(out=qb_sb, in_=qb.rearrange("(s p) o -> p s o", p=128))
            peT_sb = persist.tile([128, T], BF16)
            nc.sync.dma_start(out=peT_sb[0:DK, :], in_=peT1[:, :])
            # replicate to the upper partition half on-device (cheap SBUF DMA)
            nc.sync.dma_start(out=peT_sb[DK : 2 * DK, :], in_=peT_sb[0:DK, :])
            wvT_sb = persist.tile([128, N_CS, DO], BF16)
            nc.sync.dma_start(out=wvT_sb, in_=wvT.rearrange("(s p) i -> p s i", p=128))
            woT_sb = persist.tile([128, N_IS, D], BF16)

            ident_f32 = persist.tile([128, 128], F32)
            make_identity(nc, ident_f32)
            ident = persist.tile([128, 128], F32R)
            nc.vector.tensor_copy(out=ident, in_=ident_f32)
            ones_f32 = persist.tile([128, N_TT, HPC], F32)
            nc.vector.memset(ones_f32, 1.0)
            ident_bf = persist.tile([128, 128], BF16)
            nc.vector.tensor_copy(out=ident_bf, in_=ident_f32)
            ones_col = persist.tile([1, DK], F32R)
            eps_t = persist.tile([128, 1], F32)
            nc.vector.memset(eps_t, LN_EPS)

            xhatT = persist.tile([128, N_CS, T], BF16)  # (c, t)
            QT = persist.tile([128, N_IS, T], F32R)  # (i, t), pre-scaled by 1/8
            KT = persist.tile([128, N_IS, T], F32R)  # (i, t), pe added
            Vsb = persist.tile([128, N_TT, HPC * (DK + 1)], F32R)  # (s,[V_h|1]x4)
            ctxT = persist.tile([128, N_IS, T], BF16)  # normalized context^T

            nc.vector.tensor_copy(
                out=Vsb.rearrange("p n (h u) -> p n h u", u=DK + 1)[:, :, :, DK],
                in_=ones_f32,
            )
            nc.vector.tensor_copy(
                out=ones_col, in_=ones_f32[0:1, :, :].rearrange("p n h -> p (n h)")
            )

            # PE keep-alive: the cost model's pstate ramp needs continuous
            # PE execution (3us to full clock); idle gaps drop it to 0.65GHz
            # and triple every prologue transpose. Identity self-matmuls into
            # a reader-less psum tile keep the clock up while the LN chain
            # feeds real work.
            def pe_warm(n):
                for _ in range(n):
                    wt = ps_s.tile([128, W], F32, tag="ps", name="warm")
                    nc.tensor.matmul(wt[:, 0:128], ident_bf, ident_bf,
                                     start=True, stop=True)

            # ---- LayerNorm + transpose, one unit per x tile ----
            # Per-tile chain: stats (ACT for the earliest tiles, DVE bn for
            # the rest) -> mean/rstd arithmetic + xhat on Pool -> 4 PE
            # transposes into a [128,512] ps_mm tile -> one DVE copy into
            # xhatT. Tiles 8-15 are emitted as fillers inside the first
            # attention block so the in-order engine streams reach the first
            # scores early.
            inv_d = 1.0 / D

            def ln_unit(i):
                x_t = x_tiles[i]
                mean = lnst.tile([128, 1], F32, tag="mean")
                rstd = lnst.tile([128, 1], F32, tag="rstd")
                if i < 8 and i % 2 == 0:
                    scr = lnscr.tile([128, D], F32, tag="scr")
                    ssum = lnst.tile([128, 1], F32, tag="ssum")
                    nc.scalar.activation(
                        out=scr, in_=x_t, func=AF.Copy, accum_out=ssum
                    )
                    scr2 = lnscr.tile([128, D], F32, tag="scr")
                    ssq = lnst.tile([128, 1], F32, tag="ssq")
                    nc.scalar.activation(
                        out=scr2, in_=x_t, func=AF.Square, accum_out=ssq
                    )
                    nc.vector.tensor_scalar_mul(out=mean, in0=ssum, scalar1=inv_d)
                    vpe = lnst.tile([128, 1], F32, tag="vpe")
                    nc.vector.tensor_scalar(
                        out=vpe, in0=ssq, scalar1=inv_d, scalar2=LN_EPS,
                        op0=ALU.mult, op1=ALU.add,
                    )
                    m2 = lnst.tile([128, 1], F32, tag="m2")
                    nc.vector.tensor_mul(out=m2, in0=mean, in1=mean)
                    var = lnst.tile([128, 1], F32, tag="var")
                    nc.vector.tensor_sub(out=var, in0=vpe, in1=m2)
                    std = lnst.tile([128, 1], F32, tag="std")
                    nc.scalar.activation(out=std, in_=var, func=AF.Sqrt)
                    nc.vector.reciprocal(out=rstd, in_=std)
                else:
                    stats = lnst.tile([128, 6], F32, tag="bn")
                    nc.vector.bn_stats(out=stats, in_=x_t)
                    mv = lnst.tile([128, 2], F32, tag="mv")
                    nc.vector.bn_aggr(out=mv, in_=stats)
                    nc.vector.tensor_copy(out=mean, in_=mv[:, 0:1])
                    std = lnst.tile([128, 1], F32, tag="std")
                    nc.scalar.activation(out=std, in_=mv[:, 1:2], func=AF.Sqrt,
                                         bias=eps_t)
                    nc.vector.reciprocal(out=rstd, in_=std)
                xhat = lnw.tile([128, D], BF16, tag="xhat")
                with nc.allow_low_precision(reason="xhat to bf16 on gpsimd"):
                    nc.gpsimd.tensor_scalar(
                        out=xhat, in0=x_t, scalar1=mean, scalar2=rstd,
                        op0=ALU.subtract, op1=ALU.mult,
                    )
                nc.sync.dma_start_transpose(
                    out=xhatT[:, :, i * 128 : (i + 1) * 128], in_=xhat
                )

            # ---- projections (prologue / filler) ----
            # K feeds every attention block's full key range, so all K units
            # must precede the ss-loops that consume them; Q only feeds its
            # own t-block.
            def k_proj(isl, j):
                tj = slice(j * 512, (j + 1) * 512)
                pk = ps_mm.tile([128, 512], F32, tag="mm")
                for cs in range(N_CS):
                    nc.tensor.matmul(
                        pk,
                        wkT_sb[:, cs, isl * 128 : (isl + 1) * 128],
                        xhatT[:, cs, tj],
                        start=(cs == 0), stop=(cs == N_CS - 1),
                    )
                nc.vector.tensor_add(
                    out=KT[:, isl, tj], in0=pk, in1=peT_sb[:, tj]
                )

            def q_proj(isl, j, on_act=False):
                # qb_sb is pre-scaled by 1/sqrt(dk) on the host so the ACT
                # variant is a single Copy with scale+bias (ACT idles before
                # the first exp; keeps DVE off the first-score chain).
                tj = slice(j * 512, (j + 1) * 512)
                pq = ps_mm.tile([128, 512], F32, tag="mm")
                for cs in range(N_CS):
                    nc.tensor.matmul(
                        pq,
                        wqT_sb[:, cs, isl * 128 : (isl + 1) * 128],
                        xhatT[:, cs, tj],
                        start=(cs == 0), stop=(cs == N_CS - 1),
                    )
                if on_act:
                    nc.scalar.activation(
                        out=QT[:, isl, tj], in_=pq, func=AF.Identity,
                        scale=1.0 / math.sqrt(DK), bias=qb_sb[:, isl, :],
                    )
                else:
                    nc.vector.tensor_scalar(
                        out=QT[:, isl, tj], in0=pq,
                        scalar1=1.0 / math.sqrt(DK), scalar2=qb_sb[:, isl, :],
                        op0=ALU.mult, op1=ALU.add,
                    )

            def v_pair(st):
                # V projections for x tiles st, st+1 share one [128,512] psum
                # tile and drain with a single strided DVE copy.
                pv = ps_mm.tile([128, 2, 256], F32, tag="mm")
                for u in range(2):
                    for cs in range(N_CS):
                        nc.tensor.matmul(
                            pv[:, u, :],
                            xhatT[:, cs, (st + u) * 128 : (st + u + 1) * 128],
                            wvT_sb[:, cs, :],
                            start=(cs == 0), stop=(cs == N_CS - 1),
                        )
                nc.vector.tensor_copy(
                    out=Vsb.rearrange("p n (h u) -> p n h u", u=DK + 1)[
                        :, st : st + 2, :, 0:DK
                    ],
                    in_=pv.rearrange("p n (h u) -> p n h u", u=DK),
                )

            def attention(h, t0, w, filler=(), raw_filler=()):
                filler = list(filler)
                raw_filler = list(raw_filler)
                hp = slice((h % 2) * 64, (h % 2) * 64 + 64)
                hi = h // 2
                nhf = w // 512
                # per-hf pc tiles (1 bank each): each half drains and is
                # reusable independently, halving the WAR stall at head
                # boundaries.
                pcs = [ps_c.tile([DK + 1, 512], F32, tag="pc", name=f"pc{hf}")
                       for hf in range(nhf)]
                for ss in range(N_TT):
                    pscore = ps_s.tile([128, w], F32, tag="ps")
                    for hf in range(nhf):
                        nc.tensor.matmul(
                            pscore[:, hf * 512 : (hf + 1) * 512],
                            KT[hp, hi, ss * 128 : (ss + 1) * 128],
                            QT[hp, hi, t0 + hf * 512 : t0 + (hf + 1) * 512],
                            start=True, stop=True,
                        )
                    et = expp.tile([128, w], F32R, tag="exp")
                    nc.scalar.activation(out=et, in_=pscore, func=AF.Exp)
                    for hf in range(nhf):
                        nc.tensor.matmul(
                            pcs[hf],
                            Vsb[:, ss, h * (DK + 1) : (h + 1) * (DK + 1)],
                            et[:, hf * 512 : (hf + 1) * 512],
                            start=(ss == 0), stop=(ss == N_TT - 1),
                        )
                    if raw_filler and ss >= 1:
                        # mild depri: stay timely but yield PE slots to the
                        # score/attnV stream
                        with tc.high_priority(offset=-60):
                            raw_filler.pop(0)()
                    if filler and (ss % 2 == 1 or len(filler) > (N_TT - ss) // 2):
                        with tc.high_priority(offset=-PROJ_DEPRI):
                            filler.pop(0)()
                # pipelined per-hf drain: reciprocal of the ones-row (DVE),
                # PE broadcast of the reciprocal across 64 partitions, then
                # copy + multiply on DVE. pc is free after the copy.
                for hf in range(nhf):
                    pc = pcs[hf]
                    rrow = denp.tile([1, 512], F32R, tag="den")
                    with nc.allow_low_precision(reason="softmax recip to f32r"):
                        nc.vector.reciprocal(out=rrow, in_=pc[DK : DK + 1, :])
                    pb = ps_mm.tile([DK, 512], F32, tag="mm", name="pb")
                    nc.tensor.matmul(pb, ones_col, rrow, start=True, stop=True)
                    ctx_sl = ctxT[hp, hi, t0 + hf * 512 : t0 + (hf + 1) * 512]
                    nc.vector.tensor_copy(out=ctx_sl, in_=pc[0:DK, :])
                    nc.vector.tensor_mul(out=ctx_sl, in0=ctx_sl, in1=pb)
                for f in filler:
                    with tc.high_priority(offset=-PROJ_DEPRI):
                        f()

            def out_tile(i):
                po = ps_mm.tile([128, 512], F32, tag="mm")
                for isl in range(N_IS):
                    nc.tensor.matmul(
                        po,
                        ctxT[:, isl, i * 128 : (i + 1) * 128],
                        woT_sb[:, isl, :],
                        start=(isl == 0), stop=(isl == N_IS - 1),
                    )
                o_t = outw.tile([128, D], BF16, tag="o")
                nc.vector.tensor_copy(out=o_t, in_=po)
                nc.sync.dma_start(out=out[i * 128 : (i + 1) * 128, :], in_=o_t)

            # Final t-block's out-projection accumulates per head-group into
            # SBUF so only the h3 part remains after the last drain: h0+h1
            # (i-slab 0, 128-contraction) right after h1's drain, h2 after
            # h2's, h3 + store in the tail.
            po_parts = {}

            def out_jj1_first(i):
                po = ps_mm.tile([128, 512], F32, tag="mm")
                nc.tensor.matmul(
                    po, ctxT[:, 0, i * 128 : (i + 1) * 128], woT_sb[:, 0, :],
                    start=True, stop=True,
                )
                po_s = pop.tile([128, 512], F32, tag="po", name="po_s")
                po_parts[i] = po_s
                nc.vector.tensor_copy(out=po_s, in_=po)

            def out_jj1_mid(i):
                po = ps_mm.tile([128, 512], F32, tag="mm")
                nc.tensor.matmul(
                    po, ctxT[0:64, 1, i * 128 : (i + 1) * 128],
                    woT_sb[0:64, 1, :], start=True, stop=True,
                )
                nc.vector.tensor_add(out=po_parts[i], in0=po, in1=po_parts[i])

            def out_jj1_last(i):
                po = ps_mm.tile([128, 512], F32, tag="mm")
                nc.tensor.matmul(
                    po, ctxT[64:128, 1, i * 128 : (i + 1) * 128],
                    woT_sb[64:128, 1, :], start=True, stop=True,
                )
                o_t = outw.tile([128, D], BF16, tag="o")
                nc.vector.tensor_add(out=o_t, in0=po, in1=po_parts[i])
                nc.sync.dma_start(out=out[i * 128 : (i + 1) * 128, :], in_=o_t)

            # Prologue covers x tiles 0-7 (enough for jj0 scores: K s-tiles
            # stream per-ss, Q needs t 0-1023) plus their V tiles; tiles 8-15
            # flow as normal-priority fillers inside the first attention
            # block, one per ss step, staying 8 steps ahead of the attn@V
            # that consumes their V.
            for i in range(8, N_TT):
                nc.sync.dma_start(out=x_tiles[i], in_=xb_r[:, i, :])
            pe_warm(8)
            for i in range(12):
                ln_unit(i)
                pe_warm(6)
                if i % 2 == 1:
                    v_pair(i - 1)
                if i == 3:
                    with tc.high_priority(offset=100):
                        k_proj(0, 0)
                        q_proj(0, 0, on_act=True)
                if i == 7:
                    with tc.high_priority(offset=100):
                        k_proj(0, 1)
                        q_proj(0, 1, on_act=True)
                if i == 11:
                    with tc.high_priority(offset=100):
                        k_proj(0, 2)

            nc.sync.dma_start(
                out=woT_sb, in_=woT.rearrange("(s p) o -> p s o", p=128)
            )

            late = []
            for i in range(12, 16):
                late.append(lambda i=i: ln_unit(i))
                if i % 2 == 1:
                    late.append(lambda i=i: v_pair(i - 1))
                if i == 15:
                    late.append(lambda: k_proj(0, 3))

            attention(0, 0, W, raw_filler=late)
            attention(1, 0, W, [lambda: k_proj(1, 0), lambda: k_proj(1, 1),
                                lambda: q_proj(1, 0), lambda: q_proj(1, 1)])
            attention(2, 0, W, [lambda: k_proj(1, 2), lambda: k_proj(1, 3),
                                lambda: q_proj(0, 2), lambda: q_proj(1, 2)])
            attention(3, 0, W, [lambda: q_proj(0, 3), lambda: q_proj(1, 3)])
            attention(0, W, W, [lambda i=i: out_tile(i) for i in range(0, 3)])
            attention(1, W, W, [lambda i=i: out_tile(i) for i in range(3, 6)])
            attention(2, W, W, [lambda i=i: out_tile(i) for i in range(6, 8)]
                      + [lambda i=i: out_jj1_first(i) for i in range(8, 16)])
            attention(3, W, 512,
                      [lambda i=i: out_jj1_mid(i) for i in range(8, 16)])
            for i in range(8, 12):
                out_jj1_last(i)
            attention(3, W + 512, 512)
            for i in range(12, 16):
                out_jj1_last(i)

    split_multi_waits(nc)
    return nc


def _rel_pos_encoding_np(length: int, d: int) -> np.ndarray:
    pos = np.arange(length, dtype=np.float32)[:, None]
    div = np.exp(
        np.arange(0, d, 2, dtype=np.float32) * np.float32(-(math.log(10000.0) / d))
    ).astype(np.float32)
    ang = pos * div[None, :]
    return np.stack([np.sin(ang), np.cos(ang)], axis=-1).reshape(length, d)


def make_in_maps(x, ln_g, ln_b, wq, bq, wk, bk, wv, bv, wo, bo):
    wq_eff = (wq * ln_g[None, :]).astype(np.float32)
    wk_eff = (wk * ln_g[None, :]).astype(np.float32)
    qb_eff = ((wq_eff @ ln_b + bq) / np.sqrt(64.0)).astype(np.float32)
    wv_eff = (wv * ln_g[None, :]).astype(np.float32)
    pe = _rel_pos_encoding_np(T, DK)
    peT1 = np.ascontiguousarray(pe.T).astype(np.float32)

    import ml_dtypes
    bf = ml_dtypes.bfloat16
    in_maps = []
    for c in range(N_CORES):
        b, g = c // 2, c % 2
        hs = slice(g * DO, (g + 1) * DO)
        in_maps.append(
            {
                "xb": np.ascontiguousarray(x[b]).astype(bf),
                "wqT": np.ascontiguousarray(wq_eff[hs].T).astype(bf),
                "wkT": np.ascontiguousarray(wk_eff[hs].T).astype(bf),
                "wvT": np.ascontiguousarray(wv_eff[hs].T).astype(bf),
                "woT": np.ascontiguousarray(wo[:, hs].T).astype(bf),
                "qb": np.ascontiguousarray(qb_eff[hs].reshape(DO, 1)),
                "peT1": peT1.astype(bf),
            }
        )
    return in_maps


def host_combine(results, ln_b, wv, bv, wo, bo):
    vb_eff = wv @ ln_b + bv  # (512,)
    const_row = (vb_eff @ wo.T + bo).astype(np.float32)  # (512,)
    out = np.empty((B, T, D), dtype=np.float32)
    for b in range(B):
        out[b] = (
            np.asarray(results[2 * b]["out"], dtype=np.float32)
            + np.asarray(results[2 * b + 1]["out"], dtype=np.float32)
            + const_row
        )
    return out


def kernel(x, ln_g, ln_b, wq, bq, wk, bk, wv, bv, wo, bo, **run_kwargs):
    args = [np.asarray(a, dtype=np.float32) for a in
            (x, ln_g, ln_b, wq, bq, wk, bk, wv, bv, wo, bo)]
    x, ln_g, ln_b, wq, bq, wk, bk, wv, bv, wo, bo = args
    nc = build_nc()
    in_maps = make_in_maps(x, ln_g, ln_b, wq, bq, wk, bk, wv, bv, wo, bo)
    res = run_bass_kernel_spmd(nc, in_maps, core_ids=list(range(N_CORES)), **run_kwargs)
    out = host_combine(res.results, ln_b, wv, bv, wo, bo)
    kernel.last_results = res
    return out


# revision 3
# speedup vs baseline: 1.0258x; 1.0258x over previous
"""Trainium2 Bass kernel for MultiHeadSelfAttentionModule, v2.

Sharding: core = 2*b + g (batch b, head-group g of 4 heads). Host reduces the
two head-groups' out-proj partials per batch and folds the exact bias terms
(see kernel v1 notes; the math rewrites are unchanged).

v2 layout/schedule changes vs v1:
  - exp tiles are 1024 wide (halves ACT per-instruction overhead; ACT is the
    near-bottleneck at ~133us of exp).
  - attention runs jj-outer (t-block) / head-inner so each t-block's
    out-projection can start right after its 4 heads finish, shrinking the
    epilogue tail to the last block only.
  - all Q/K/V projections are emitted as prologue/filler work; PE slack
    during the ACT-paced attention stream absorbs them.
  - softmax normalization: the V-ones row gives the denominator row in PSUM;
    DVE copies it to SBUF, GPSIMD partition_broadcast replicates it across 64
    partitions, and a single DVE tensor_tensor(divide) writes normalized
    context straight to ctxT. No PE broadcast matmuls, no extra PSUM bank.
  - LN stats: ACT Copy/Square+accum for the first tiles (ACT idles before the
    first exp), DVE bn_stats for the rest; rstd = (var+eps)^-0.5 via a single
    DVE tensor_scalar pow (ACT Sqrt no longer needed).
  - x.T transposes go through one [128,512] PSUM tile (4 PE transposes) and
    drain with a single strided DVE copy per x-tile.
  - out is written bf16 (halves the out DMA and the tail); host upcasts.

PSUM budget: ps_s 2x[128,1024] (4 banks) + ps_c 1x[65,1024] (2 banks) +
ps_mm 2x[128,512] (2 banks) = 8 banks exactly.
"""

import math
import sys

if "/opt/trn_rl_repo" not in sys.path:
    sys.path.insert(0, "/opt/trn_rl_repo")

import numpy as np

import concourse.bass as bass
import concourse.mybir as mybir
import concourse.tile as tile
from concourse.bass_utils import run_bass_kernel_spmd
from concourse.masks import make_identity

B, T, D = 4, 2048, 512
H, DK = 8, 64
HPC = 4  # heads per core
DO = HPC * DK  # per-core head dims = 256
N_CORES = 8
LN_EPS = 1e-5
F32 = mybir.dt.float32
F32R = mybir.dt.float32r
BF16 = mybir.dt.bfloat16
AF = mybir.ActivationFunctionType
ALU = mybir.AluOpType

N_TT = T // 128  # 16 t tiles
N_CS = D // 128  # 4 contraction slabs
N_IS = DO // 128  # 2 own-dim slabs
W = 1024  # exp block width
N_JJ = T // W  # 2
ACT_STATS = 10  # below this, even tiles' stats run on ACT (odd/rest: DVE bn)
PROJ_DEPRI = 300


def split_multi_waits(nc: bass.Bass) -> None:
    """Hoist all-but-one sync wait from every instruction onto injected
    single-wait NOPs on the same engine, immediately before the owner."""
    ctr = 0
    for fn in nc.m.functions:
        for bb in fn.blocks:
            insts = bb.instructions
            need = any(
                i.sync_info and i.sync_info.on_wait and len(i.sync_info.on_wait) > 1
                for i in insts
            )
            if not need:
                continue
            new = []
            for inst in insts:
                si = inst.sync_info
                if si and si.on_wait and len(si.on_wait) > 1:
                    waits = list(si.on_wait)
                    for w in waits[:-1]:
                        ctr += 1
                        nop = mybir.InstNoOp(
                            name=f"I-wsplit-{ctr}",
                            engine=inst.engine,
                            sync_info=mybir.SyncInfo(on_wait=[w], on_update=[]),
                        )
                        nc.register_instruction(nop)
                        new.append(nop)
                    si.on_wait = [waits[-1]]
                new.append(inst)
            bb.instructions = new


def build_nc() -> bass.Bass:
    nc = bass.Bass()

    xb = nc.declare_dram_parameter("xb", [T, D], BF16, isOutput=False)
    wqT = nc.declare_dram_parameter("wqT", [D, DO], BF16, isOutput=False)
    wkT = nc.declare_dram_parameter("wkT", [D, DO], BF16, isOutput=False)
    wvT = nc.declare_dram_parameter("wvT", [D, DO], BF16, isOutput=False)
    woT = nc.declare_dram_parameter("woT", [DO, D], BF16, isOutput=False)
    qb = nc.declare_dram_parameter("qb", [DO, 1], F32, isOutput=False)
    peT1 = nc.declare_dram_parameter("peT1", [DK, T], BF16, isOutput=False)
    out = nc.declare_dram_parameter("out", [T, D], BF16, isOutput=True)

    with tile.TileContext(nc) as tc:
        with (
            tc.tile_pool(name="persist", bufs=1) as persist,
            tc.tile_pool(name="xstream", bufs=16) as xstream,
            tc.tile_pool(name="lnscr", bufs=4) as lnscr,
            tc.tile_pool(name="lnst", bufs=16) as lnst,
            tc.tile_pool(name="lnw", bufs=6) as lnw,
            tc.tile_pool(name="expp", bufs=8) as expp,
            tc.tile_pool(name="denp", bufs=2) as denp,
            tc.tile_pool(name="pbp", bufs=2) as pbp,
            tc.tile_pool(name="outw", bufs=3) as outw,
            tc.tile_pool(name="pop", bufs=8) as pop,
            tc.tile_pool(name="ps_s", bufs=2, space="PSUM") as ps_s,
            tc.tile_pool(name="ps_c", bufs=2, space="PSUM") as ps_c,
            tc.tile_pool(name="ps_mm", bufs=2, space="PSUM") as ps_mm,
        ):
            xb_r = xb.rearrange("(n p) d -> p n d", p=128)
            x_tiles = [
                xstream.tile([128, D], BF16, tag="x", name=f"x_t{i}")
                for i in range(N_TT)
            ]
            for i in range(8):
                nc.sync.dma_start(out=x_tiles[i], in_=xb_r[:, i, :])

            # k/q weights + pe land right after the first 8 x tiles so the
            # i-slab-0 projections can start while x8-15 stream. pe is stored
            # [64, T] in DRAM and replicated to both partition halves (all
            # heads share it; both i-slabs read the same [128, T] tile).
            wkT_sb = persist.tile([128, N_CS, DO], BF16)
            nc.sync.dma_start(out=wkT_sb, in_=wkT.rearrange("(s p) i -> p s i", p=128))
            wqT_sb = persist.tile([128, N_CS, DO], BF16)
            nc.sync.dma_start(out=wqT_sb, in_=wqT.rearrange("(s p) i -> p s i", p=128))
            qb_sb = persist.tile([128, N_IS, 1], F32)
            nc.sync.dma_start(out=qb_sb, in_=qb.rearrange("(s p) o -> p s o", p=128))
            peT_sb = persist.tile([128, T], BF16)
            nc.sync.dma_start(out=peT_sb[0:DK, :], in_=peT1[:, :])
            # replicate to the upper partition half on-device (cheap SBUF DMA)
            nc.sync.dma_start(out=peT_sb[DK : 2 * DK, :], in_=peT_sb[0:DK, :])
            wvT_sb = persist.tile([128, N_CS, DO], BF16)
            nc.sync.dma_start(out=wvT_sb, in_=wvT.rearrange("(s p) i -> p s i", p=128))
            woT_sb = persist.tile([128, N_IS, D], BF16)

            ident_f32 = persist.tile([128, 128], F32)
            make_identity(nc, ident_f32)
            ident = persist.tile([128, 128], F32R)
            nc.vector.tensor_copy(out=ident, in_=ident_f32)
            ones_f32 = persist.tile([128, N_TT, HPC], F32)
            nc.vector.memset(ones_f32, 1.0)
            ident_bf = persist.tile([128, 128], BF16)
            nc.vector.tensor_copy(out=ident_bf, in_=ident_f32)
            ones_col = persist.tile([1, DK], F32R)
            eps_t = persist.tile([128, 1], F32)
            nc.vector.memset(eps_t, LN_EPS)

            xhatT = persist.tile([128, N_CS, T], BF16)  # (c, t)
            QT = persist.tile([128, N_IS, T], F32R)  # (i, t), pre-scaled by 1/8
            KT = persist.tile([128, N_IS, T], F32R)  # (i, t), pe added
            Vsb = persist.tile([128, N_TT, HPC * (DK + 1)], F32R)  # (s,[V_h|1]x4)
            ctxT = persist.tile([128, N_IS, T], BF16)  # normalized context^T

            nc.vector.tensor_copy(
                out=Vsb.rearrange("p n (h u) -> p n h u", u=DK + 1)[:, :, :, DK],
                in_=ones_f32,
            )
            nc.vector.tensor_copy(
                out=ones_col, in_=ones_f32[0:1, :, :].rearrange("p n h -> p (n h)")
            )

            # PE keep-alive: the cost model's pstate ramp needs continuous
            # PE execution (3us to full clock); idle gaps drop it to 0.65GHz
            # and triple every prologue transpose. Identity self-matmuls into
            # a reader-less psum tile keep the clock up while the LN chain
            # feeds real work.
            def pe_warm(n):
                for _ in range(n):
                    wt = ps_s.tile([128, W], F32, tag="ps", name="warm")
                    nc.tensor.matmul(wt[:, 0:128], ident_bf, ident_bf,
                                     start=True, stop=True)

            # ---- LayerNorm + transpose, one unit per x tile ----
            # Per-tile chain: stats (ACT for the earliest tiles, DVE bn for
            # the rest) -> mean/rstd arithmetic + xhat on Pool -> 4 PE
            # transposes into a [128,512] ps_mm tile -> one DVE copy into
            # xhatT. Tiles 8-15 are emitted as fillers inside the first
            # attention block so the in-order engine streams reach the first
            # scores early.
            inv_d = 1.0 / D

            def ln_unit(i):
                x_t = x_tiles[i]
                mean = lnst.tile([128, 1], F32, tag="mean")
                rstd = lnst.tile([128, 1], F32, tag="rstd")
                if i < 8 and i % 2 == 0:
                    scr = lnscr.tile([128, D], F32, tag="scr")
                    ssum = lnst.tile([128, 1], F32, tag="ssum")
                    nc.scalar.activation(
                        out=scr, in_=x_t, func=AF.Copy, accum_out=ssum
                    )
                    scr2 = lnscr.tile([128, D], F32, tag="scr")
                    ssq = lnst.tile([128, 1], F32, tag="ssq")
                    nc.scalar.activation(
                        out=scr2, in_=x_t, func=AF.Square, accum_out=ssq
                    )
                    nc.vector.tensor_scalar_mul(out=mean, in0=ssum, scalar1=inv_d)
                    vpe = lnst.tile([128, 1], F32, tag="vpe")
                    nc.vector.tensor_scalar(
                        out=vpe, in0=ssq, scalar1=inv_d, scalar2=LN_EPS,
                        op0=ALU.mult, op1=ALU.add,
                    )
                    m2 = lnst.tile([128, 1], F32, tag="m2")
                    nc.vector.tensor_mul(out=m2, in0=mean, in1=mean)
                    var = lnst.tile([128, 1], F32, tag="var")
                    nc.vector.tensor_sub(out=var, in0=vpe, in1=m2)
                    std = lnst.tile([128, 1], F32, tag="std")
                    nc.scalar.activation(out=std, in_=var, func=AF.Sqrt)
                    nc.vector.reciprocal(out=rstd, in_=std)
                else:
                    stats = lnst.tile([128, 6], F32, tag="bn")
                    nc.vector.bn_stats(out=stats, in_=x_t)
                    mv = lnst.tile([128, 2], F32, tag="mv")
                    nc.vector.bn_aggr(out=mv, in_=stats)
                    nc.vector.tensor_copy(out=mean, in_=mv[:, 0:1])
                    std = lnst.tile([128, 1], F32, tag="std")
                    nc.scalar.activation(out=std, in_=mv[:, 1:2], func=AF.Sqrt,
                                         bias=eps_t)
                    nc.vector.reciprocal(out=rstd, in_=std)
                xhat = lnw.tile([128, D], BF16, tag="xhat")
                with nc.allow_low_precision(reason="xhat to bf16 on gpsimd"):
                    nc.gpsimd.tensor_scalar(
                        out=xhat, in0=x_t, scalar1=mean, scalar2=rstd,
                        op0=ALU.subtract, op1=ALU.mult,
                    )
                nc.sync.dma_start_transpose(
                    out=xhatT[:, :, i * 128 : (i + 1) * 128], in_=xhat
                )

            # ---- projections (prologue / filler) ----
            # K feeds every attention block's full key range, so all K units
            # must precede the ss-loops that consume them; Q only feeds its
            # own t-block.
            def k_proj(isl, j):
                tj = slice(j * 512, (j + 1) * 512)
                pk = ps_mm.tile([128, 512], F32, tag="mm")
                for cs in range(N_CS):
                    nc.tensor.matmul(
                        pk,
                        wkT_sb[:, cs, isl * 128 : (isl + 1) * 128],
                        xhatT[:, cs, tj],
                        start=(cs == 0), stop=(cs == N_CS - 1),
                    )
                nc.vector.tensor_add(
                    out=KT[:, isl, tj], in0=pk, in1=peT_sb[:, tj]
                )

            def q_proj(isl, j, on_act=False):
                # qb_sb is pre-scaled by 1/sqrt(dk) on the host so the ACT
                # variant is a single Copy with scale+bias (ACT idles before
                # the first exp; keeps DVE off the first-score chain).
                tj = slice(j * 512, (j + 1) * 512)
                pq = ps_mm.tile([128, 512], F32, tag="mm")
                for cs in range(N_CS):
                    nc.tensor.matmul(
                        pq,
                        wqT_sb[:, cs, isl * 128 : (isl + 1) * 128],
                        xhatT[:, cs, tj],
                        start=(cs == 0), stop=(cs == N_CS - 1),
                    )
                if on_act:
                    nc.scalar.activation(
                        out=QT[:, isl, tj], in_=pq, func=AF.Identity,
                        scale=1.0 / math.sqrt(DK), bias=qb_sb[:, isl, :],
                    )
                else:
                    nc.vector.tensor_scalar(
                        out=QT[:, isl, tj], in0=pq,
                        scalar1=1.0 / math.sqrt(DK), scalar2=qb_sb[:, isl, :],
                        op0=ALU.mult, op1=ALU.add,
                    )

            def v_pair(st):
                # V projections for x tiles st, st+1 share one [128,512] psum
                # tile and drain with a single strided DVE copy.
                pv = ps_mm.tile([128, 2, 256], F32, tag="mm")
                for u in range(2):
                    for cs in range(N_CS):
                        nc.tensor.matmul(
                            pv[:, u, :],
                            xhatT[:, cs, (st + u) * 128 : (st + u + 1) * 128],
                            wvT_sb[:, cs, :],
                            start=(cs == 0), stop=(cs == N_CS - 1),
                        )
                nc.vector.tensor_copy(
                    out=Vsb.rearrange("p n (h u) -> p n h u", u=DK + 1)[
                        :, st : st + 2, :, 0:DK
                    ],
                    in_=pv.rearrange("p n (h u) -> p n h u", u=DK),
                )

            def attention(h, t0, w, filler=(), raw_filler=()):
                filler = list(filler)
                raw_filler = list(raw_filler)
                hp = slice((h % 2) * 64, (h % 2) * 64 + 64)
                hi = h // 2
                nhf = w // 512
                # per-hf pc tiles (1 bank each): each half drains and is
                # reusable independently, halving the WAR stall at head
                # boundaries.
                pcs = [ps_c.tile([DK + 1, 512], F32, tag="pc", name=f"pc{hf}")
                       for hf in range(nhf)]
                for ss in range(N_TT):
                    pscore = ps_s.tile([128, w], F32, tag="ps")
                    for hf in range(nhf):
                        nc.tensor.matmul(
                            pscore[:, hf * 512 : (hf + 1) * 512],
                            KT[hp, hi, ss * 128 : (ss + 1) * 128],
                            QT[hp, hi, t0 + hf * 512 : t0 + (hf + 1) * 512],
                            start=True, stop=True,
                        )
                    et = expp.tile([128, w], F32R, tag="exp")
                    nc.scalar.activation(out=et, in_=pscore, func=AF.Exp)
                    for hf in range(nhf):
                        nc.tensor.matmul(
                            pcs[hf],
                            Vsb[:, ss, h * (DK + 1) : (h + 1) * (DK + 1)],
                            et[:, hf * 512 : (hf + 1) * 512],
                            start=(ss == 0), stop=(ss == N_TT - 1),
                        )
                    if raw_filler and ss >= 1:
                        # mild depri: stay timely but yield PE slots to the
                        # score/attnV stream
                        with tc.high_priority(offset=-60):
                            raw_filler.pop(0)()
                    if filler and (ss % 2 == 1 or len(filler) > (N_TT - ss) // 2):
                        with tc.high_priority(offset=-PROJ_DEPRI):
                            filler.pop(0)()
                # pipelined per-hf drain: reciprocal of the ones-row (DVE),
                # PE broadcast of the reciprocal across 64 partitions, then
                # copy + multiply on DVE. pc is free after the copy.
                drains = tc.high_priority(offset=-60)
                drains.__enter__()
                for hf in range(nhf):
                    pc = pcs[hf]
                    rrow = denp.tile([1, 512], F32R, tag="den")
                    with nc.allow_low_precision(reason="softmax recip to f32r"):
                        nc.vector.reciprocal(out=rrow, in_=pc[DK : DK + 1, :])
                    pb = ps_mm.tile([DK, 512], F32, tag="mm", name="pb")
                    nc.tensor.matmul(pb, ones_col, rrow, start=True, stop=True)
                    ctx_sl = ctxT[hp, hi, t0 + hf * 512 : t0 + (hf + 1) * 512]
                    nc.vector.tensor_copy(out=ctx_sl, in_=pc[0:DK, :])
                    nc.vector.tensor_mul(out=ctx_sl, in0=ctx_sl, in1=pb)
                drains.__exit__(None, None, None)
                for f in filler:
                    with tc.high_priority(offset=-PROJ_DEPRI):
                        f()

            def out_tile(i):
                po = ps_mm.tile([128, 512], F32, tag="mm")
                for isl in range(N_IS):
                    nc.tensor.matmul(
                        po,
                        ctxT[:, isl, i * 128 : (i + 1) * 128],
                        woT_sb[:, isl, :],
                        start=(isl == 0), stop=(isl == N_IS - 1),
                    )
                o_t = outw.tile([128, D], BF16, tag="o")
                nc.vector.tensor_copy(out=o_t, in_=po)
                nc.sync.dma_start(out=out[i * 128 : (i + 1) * 128, :], in_=o_t)

            # Final t-block's out-projection accumulates per head-group into
            # SBUF so only the h3 part remains after the last drain: h0+h1
            # (i-slab 0, 128-contraction) right after h1's drain, h2 after
            # h2's, h3 + store in the tail.
            po_parts = {}

            def out_jj1_first(i):
                po = ps_mm.tile([128, 512], F32, tag="mm")
                nc.tensor.matmul(
                    po, ctxT[:, 0, i * 128 : (i + 1) * 128], woT_sb[:, 0, :],
                    start=True, stop=True,
                )
                po_s = pop.tile([128, 512], F32, tag="po", name="po_s")
                po_parts[i] = po_s
                nc.vector.tensor_copy(out=po_s, in_=po)

            def out_jj1_mid(i):
                po = ps_mm.tile([128, 512], F32, tag="mm")
                nc.tensor.matmul(
                    po, ctxT[0:64, 1, i * 128 : (i + 1) * 128],
                    woT_sb[0:64, 1, :], start=True, stop=True,
                )
                nc.vector.tensor_add(out=po_parts[i], in0=po, in1=po_parts[i])

            def out_jj1_last(i):
                po = ps_mm.tile([128, 512], F32, tag="mm")
                nc.tensor.matmul(
                    po, ctxT[64:128, 1, i * 128 : (i + 1) * 128],
                    woT_sb[64:128, 1, :], start=True, stop=True,
                )
                o_t = outw.tile([128, D], BF16, tag="o")
                nc.vector.tensor_add(out=o_t, in0=po, in1=po_parts[i])
                nc.sync.dma_start(out=out[i * 128 : (i + 1) * 128, :], in_=o_t)

            # Prologue covers x tiles 0-7 (enough for jj0 scores: K s-tiles
            # stream per-ss, Q needs t 0-1023) plus their V tiles; tiles 8-15
            # flow as normal-priority fillers inside the first attention
            # block, one per ss step, staying 8 steps ahead of the attn@V
            # that consumes their V.
            for i in range(8, N_TT):
                nc.sync.dma_start(out=x_tiles[i], in_=xb_r[:, i, :])
            pe_warm(8)
            for i in range(12):
                ln_unit(i)
                pe_warm(6)
                if i % 2 == 1:
                    v_pair(i - 1)
                if i == 3:
                    with tc.high_priority(offset=100):
                        k_proj(0, 0)
                        q_proj(0, 0, on_act=True)
                if i == 7:
                    with tc.high_priority(offset=100):
                        k_proj(0, 1)
                        q_proj(0, 1, on_act=True)
                if i == 11:
                    with tc.high_priority(offset=100):
                        k_proj(0, 2)

            nc.sync.dma_start(
                out=woT_sb, in_=woT.rearrange("(s p) o -> p s o", p=128)
            )

            late = []
            for i in range(12, 16):
                late.append(lambda i=i: ln_unit(i))
                if i % 2 == 1:
                    late.append(lambda i=i: v_pair(i - 1))
                if i == 15:
                    late.append(lambda: k_proj(0, 3))

            attention(0, 0, W, raw_filler=late)
            attention(1, 0, W, [lambda: k_proj(1, 0), lambda: k_proj(1, 1),
                                lambda: q_proj(1, 0), lambda: q_proj(1, 1)])
            attention(2, 0, W, [lambda: k_proj(1, 2), lambda: k_proj(1, 3),
                                lambda: q_proj(0, 2), lambda: q_proj(1, 2)])
            attention(3, 0, W, [lambda: q_proj(0, 3), lambda: q_proj(1, 3)])
            attention(0, W, W, [lambda i=i: out_tile(i) for i in range(0, 3)])
            attention(1, W, W, [lambda i=i: out_tile(i) for i in range(3, 6)])
            attention(2, W, W, [lambda i=i: out_tile(i) for i in range(6, 8)]
                      + [lambda i=i: out_jj1_first(i) for i in range(8, 16)])
            attention(3, W, 512,
                      [lambda i=i: out_jj1_mid(i) for i in range(8, 16)])
            for i in range(8, 12):
                out_jj1_last(i)
            attention(3, W + 512, 512)
            for i in range(12, 16):
                out_jj1_last(i)

    split_multi_waits(nc)
    return nc


def _rel_pos_encoding_np(length: int, d: int) -> np.ndarray:
    pos = np.arange(length, dtype=np.float32)[:, None]
    div = np.exp(
        np.arange(0, d, 2, dtype=np.float32) * np.float32(-(math.log(10000.0) / d))
    ).astype(np.float32)
    ang = pos * div[None, :]
    return np.stack([np.sin(ang), np.cos(ang)], axis=-1).reshape(length, d)


def make_in_maps(x, ln_g, ln_b, wq, bq, wk, bk, wv, bv, wo, bo):
    wq_eff = (wq * ln_g[None, :]).astype(np.float32)
    wk_eff = (wk * ln_g[None, :]).astype(np.float32)
    qb_eff = ((wq_eff @ ln_b + bq) / np.sqrt(64.0)).astype(np.float32)
    wv_eff = (wv * ln_g[None, :]).astype(np.float32)
    pe = _rel_pos_encoding_np(T, DK)
    peT1 = np.ascontiguousarray(pe.T).astype(np.float32)

    import ml_dtypes
    bf = ml_dtypes.bfloat16
    in_maps = []
    for c in range(N_CORES):
        b, g = c // 2, c % 2
        hs = slice(g * DO, (g + 1) * DO)
        in_maps.append(
            {
                "xb": np.ascontiguousarray(x[b]).astype(bf),
                "wqT": np.ascontiguousarray(wq_eff[hs].T).astype(bf),
                "wkT": np.ascontiguousarray(wk_eff[hs].T).astype(bf),
                "wvT": np.ascontiguousarray(wv_eff[hs].T).astype(bf),
                "woT": np.ascontiguousarray(wo[:, hs].T).astype(bf),
                "qb": np.ascontiguousarray(qb_eff[hs].reshape(DO, 1)),
                "peT1": peT1.astype(bf),
            }
        )
    return in_maps


def host_combine(results, ln_b, wv, bv, wo, bo):
    vb_eff = wv @ ln_b + bv  # (512,)
    const_row = (vb_eff @ wo.T + bo).astype(np.float32)  # (512,)
    out = np.empty((B, T, D), dtype=np.float32)
    for b in range(B):
        out[b] = (
            np.asarray(results[2 * b]["out"], dtype=np.float32)
            + np.asarray(results[2 * b + 1]["out"], dtype=np.float32)
            + const_row
        )
    return out


def kernel(x, ln_g, ln_b, wq, bq, wk, bk, wv, bv, wo, bo, **run_kwargs):
    args = [np.asarray(a, dtype=np.float32) for a in
            (x, ln_g, ln_b, wq, bq, wk, bk, wv, bv, wo, bo)]
    x, ln_g, ln_b, wq, bq, wk, bk, wv, bv, wo, bo = args
    nc = build_nc()
    in_maps = make_in_maps(x, ln_g, ln_b, wq, bq, wk, bk, wv, bv, wo, bo)
    res = run_bass_kernel_spmd(nc, in_maps, core_ids=list(range(N_CORES)), **run_kwargs)
    out = host_combine(res.results, ln_b, wv, bv, wo, bo)
    kernel.last_results = res
    return out
